# revision 65
# baseline (speedup 1.0000x reference)
"""Trainium2 Bass kernel for nn_Attention_6468220748045.

Computes, per batch item: QKV projection -> per-head scaled attention with a
multiplicative positional bias w[i,j] = |i-j|/S -> softmax -> attn @ V ->
LayerNorm over the embedding dim.

Sharding: pure data-parallel over batch. B=128 splits as 16 batch items per
core across 8 NeuronCores; no collectives needed. Inputs are pre-laid-out on
host: x is passed transposed per batch ([B, E, S]) so both projection
orientations stream directly from SBUF, and the weights are passed transposed
([e_in, e_out]) to serve as matmul stationary operands.

Schedule (from HW trace analysis): the attention matmuls (scores N=179,
PV N=65) are LDWEIGHTS-bound (~95ns per stationary load vs 27-75ns streams)
while projection matmuls are stream-bound (149-213ns) with weight loads fully
hidden.  So projections of pair p+1 interleave 1:1 at single-matmul
granularity into the attention of pair p, hiding attention LDWEIGHTS under
projection streams.  Within a batch, PV lags scores by two heads so the
softmax chain (VectorE w-mul -> ScalarE exp) always completes well before
the PE reaches the PV matmuls (the PE queue is strictly in-order; a stalled
head-of-line matmul starves everything, including the HAM clock gate).

Engine placement: all PSUM->SBUF evacuations (q/k/v projections) run on
ScalarE via activation-Copy; ScalarE otherwise only runs Exp, and Copy lives
in every ACT table, so the table loads exactly once (v1 paid 20 x 1.3us
swapping Exp<->Sqrt).  LayerNorm rstd uses a bit-trick + Newton rsqrt on
VectorE (no table function).  GpSimd handles the small memsets.

Known remaining headroom (designed, unlanded -- see below; plus two blocked
items): (a) ~50us: attention LDWEIGHTS serialize with their matmuls because
this toolchain hardcodes --enable-ldw-opt=false (no background weight-buffer
overlap); (b) ~13us: the projection-less final pair runs at HAM half-clock --
experimentally unfixable by scheduling (chain-bound slots cannot sustain the
clock gate's array-busy threshold).  (c) ~27us, IMPLEMENTABLE: offset-
partition V-tail packing.  The two s=128:179 V-projection stationary groups
per pair (51 useful rows each) can merge into ONE group via a strided
stationary [b0-tail 51 | 13 pad | b1-tail 51] IF batch-1's jt=1 data lives at
partitions 64:115 end-to-end: V-psum rows then land at 0:51 / 64:115 so both
evacuations are partition-ALIGNED (the usual partition-shift blocker
vanishes); scores jt=1 for batch 1 writes out=ps[64:115] (legal: output base
partition 64 is col-group aligned for <=64-row outputs); PV jt=1 for batch 1
slices both operands at base partition 64 (bases match, tile_position=(64,0)
legal); requires a second wsc plane with jt=1 rows at 64:115 and padding the
xt tile free dim by 13 cols so the strided stationary's pad-read cannot
overrun the tile at k=KT-1 (race-detector clean).  Cuts V-proj from 64 to 48
N=512 matmuls per pair.
"""

import numpy as np

import concourse.bass as bass
import concourse.tile as tile
from concourse import bacc, mybir
from concourse.bass_utils import run_bass_kernel_spmd

# Problem constants (hardcoded per the self-contained-kernel contract).
B, S, E, H, D = 128, 179, 1024, 16, 64
NCORES = 8
BPC = B // NCORES          # batches per core = 16
NPAIR = BPC // 2           # batch pairs per core = 8
KT = E // 128              # contraction tiles over e_in = 8
MT = E // 128              # output tiles over e_out = 8
S0 = 128                   # first s-tile size
S1 = S - S0                # second s-tile size = 51
SP = 192                   # padded per-batch s stride in xt (V-tail packing)
S_TILES = ((0, S0), (S0, S1))
LN_EPS = 1e-5
SCALE = float(E) ** -0.5
PV_LAG = 2                 # heads by which PV trails scores

F32 = mybir.dt.float32
BF16 = mybir.dt.bfloat16
FP16 = mybir.dt.float16
U32 = mybir.dt.uint32

AF = mybir.ActivationFunctionType
ALU = mybir.AluOpType


def _build_kernel(bpc: int = BPC, apply_gb: bool = True) -> bass.Bass:
    npair = bpc // 2
    nc = bacc.Bacc()

    xT = nc.dram_tensor("xT", [bpc, E, S], BF16, kind="ExternalInput").ap()
    wqT = nc.dram_tensor("wqT", [E, E], BF16, kind="ExternalInput").ap()
    wkT = nc.dram_tensor("wkT", [E, E], BF16, kind="ExternalInput").ap()
    wvT = nc.dram_tensor("wvT", [E, E], BF16, kind="ExternalInput").ap()
    wsc = nc.dram_tensor("wsc", [128, 4, S], F32, kind="ExternalInput").ap()
    gamma = nc.dram_tensor("gamma", [E], F32, kind="ExternalInput").ap()
    beta = nc.dram_tensor("beta", [E], F32, kind="ExternalInput").ap()
    out = nc.dram_tensor("out", [bpc, S, E], F32, kind="ExternalOutput").ap()

    with tile.TileContext(nc) as tc:
        _emit(tc, npair, out, xT, wqT, wkT, wvT, wsc, gamma, beta, apply_gb)
    nc.compile()
    return nc


def _emit(tc, npair, out, xT, wqT, wkT, wvT, wsc, gamma, beta, apply_gb):
    nc = tc.nc
    from contextlib import ExitStack

    with ExitStack() as ctx:
        singles = ctx.enter_context(tc.tile_pool(name="singles", bufs=1))
        xt_pool = ctx.enter_context(tc.tile_pool(name="xt", bufs=3))
        qk_pool = ctx.enter_context(tc.tile_pool(name="qk", bufs=3))
        v_pool = ctx.enter_context(tc.tile_pool(name="v", bufs=6))
        p_pool = ctx.enter_context(tc.tile_pool(name="p", bufs=8))
        o_pool = ctx.enter_context(tc.tile_pool(name="o", bufs=4))
        ln_pool = ctx.enter_context(tc.tile_pool(name="ln", bufs=4))
        r_pool = ctx.enter_context(tc.tile_pool(name="r", bufs=8))
        s_pool = ctx.enter_context(tc.tile_pool(name="s", bufs=4))

        # PSUM (8 banks, every tile pads to one bank):
        # proj 3 + scores 3 + PV 2 = 8.  Scores pairs use 2 banks at a time;
        # the third drains through the mul/exp chain of the previous pair.
        pp_proj = ctx.enter_context(tc.tile_pool(name="pp_proj", bufs=2, space="PSUM"))
        pp_s = ctx.enter_context(tc.tile_pool(name="pp_s", bufs=4, space="PSUM"))
        pp_o = ctx.enter_context(tc.tile_pool(name="pp_o", bufs=2, space="PSUM"))

        # --- resident tensors -------------------------------------------------
        # Weight tiles: [e_in partition, k-tile, e_out]. DMA order matters for
        # startup latency: wq first, then pair-0's x.T, then wk/wv.
        xsrc = xT.rearrange("b (k p) s -> k p b s", p=128)  # [KT, 128, bpc, S]
        w_sbs = []
        for name in ("wq", "wk", "wv"):
            w_sb = singles.tile([128, KT, E], BF16, tag=f"w_{name}", name=f"w_{name}")
            w_sbs.append(w_sb)
        wq_sb, wk_sb, wv_sb = w_sbs
        # The merged V-tail stationary reads a packed [b0-tail | b1-tail]
        # tile ([128, KT, 2, 64]: 51 data + 13 zero-pad cols per batch) so
        # its psum rows land at 0:51 / 64:115; the tails are DMA'd a second
        # time from DRAM in this layout.
        xt0 = xt_pool.tile([128, KT, 2, S], BF16, tag="xt", name="xt_0")
        xtail0 = xt_pool.tile([128, KT, 2, 64], BF16, tag="xtail", name="xtail_0")
        nc.gpsimd.memset(xtail0[:, :, :, S1:64], 0.0)
        src = wqT.rearrange("(k p) e -> k p e", p=128)
        for k in range(KT):
            nc.sync.dma_start(out=wq_sb[:, k], in_=src[k])
            nc.sync.dma_start(out=xt0[:, k], in_=xsrc[k, :, 0:2, :])
            nc.sync.dma_start(out=xtail0[:, k, :, 0:S1], in_=xsrc[k, :, 0:2, 128:S])
        for w_sb, wap in ((wk_sb, wkT), (wv_sb, wvT)):
            src = wap.rearrange("(k p) e -> k p e", p=128)
            for k in range(KT):
                nc.sync.dma_start(out=w_sb[:, k], in_=src[k])

        # Positional bias (already includes softmax scale), host-precomputed.
        # Planes (0,1) serve batch 0 of a pair ([j mod 128, jt, i], zero rows
        # for j >= S); planes (2,3) serve batch 1, whose jt=1 rows live at
        # partitions 64:115 (V-tail packing layout).
        wsc_sb = singles.tile([128, 4, S], F32, tag="wsc")
        nc.sync.dma_start(out=wsc_sb, in_=wsc)

        if apply_gb:
            gamma_b = singles.tile([128, E], F32, tag="gamma")
            beta_b = singles.tile([128, E], F32, tag="beta")
            nc.sync.dma_start(
                out=gamma_b,
                in_=bass.AP(tensor=gamma.tensor, offset=gamma.offset, ap=[[0, 128]] + gamma.ap),
            )
            nc.sync.dma_start(
                out=beta_b,
                in_=bass.AP(tensor=beta.tensor, offset=beta.offset, ap=[[0, 128]] + beta.ap),
            )
        # Magic constant for the bit-trick rsqrt seed (no ACT table needed).
        magic_t = singles.tile([128, 1], U32, tag="magic")
        nc.vector.memset(magic_t, 0x5F3759DF)

        # Per-pair SBUF products handed from the projection stage to the
        # attention stage (software pipeline).  stage_qk lands after the QK
        # phase; stage_v[(pr, bi)] after that batch's V chunks, so the last
        # pair's V work can interleave into its own attention phase.
        stage_qk: dict = {}
        stage_v: dict = {}

        def proj_gen(pr):
            """QKV projections for batch pair `pr`; yields after each PE
            psum-group (~8 matmuls) so attention of pair pr-1 interleaves at
            chunk granularity (the PE weight-load path is one-deep, so finer
            interleave only serializes LDWEIGHTS of the two streams)."""
            if pr == 0:
                xt, xtail = xt0, xtail0
            else:
                xt = xt_pool.tile([128, KT, 2, S], BF16, tag="xt", name=f"xt_{pr}")
                xtail = xt_pool.tile(
                    [128, KT, 2, 64], BF16, tag="xtail", name=f"xtail_{pr}"
                )
                nc.gpsimd.memset(xtail[:, :, :, S1:64], 0.0)
                for k in range(KT):
                    nc.sync.dma_start(
                        out=xt[:, k], in_=xsrc[k, :, 2 * pr : 2 * pr + 2, :]
                    )
                    nc.sync.dma_start(
                        out=xtail[:, k, :, 0:S1],
                        in_=xsrc[k, :, 2 * pr : 2 * pr + 2, 128:S],
                    )

            # Q.T / K.T: out[e_out, s2], s2 = 2*S = 358 (both batches at once).
            qt_sb = qk_pool.tile([128, MT, 2, S], BF16, tag="qt", name=f"qt_{pr}")
            kt_sb = qk_pool.tile([128, MT, 2, S], BF16, tag="kt", name=f"kt_{pr}")
            for wi, (w_sb, dst) in enumerate(((wq_sb, qt_sb), (wk_sb, kt_sb))):
                for m in range(MT):
                    ps = pp_proj.tile([128, 2, S], F32, tag="proj", name=f"psqk_{pr}_{m}")
                    for k in range(KT):
                        nc.tensor.matmul(
                            out=ps,
                            lhsT=w_sb[:, k, m * 128 : (m + 1) * 128],
                            rhs=xt[:, k],
                            start=(k == 0),
                            stop=(k == KT - 1),
                        )
                    # PSUM->SBUF evacuation alternating VectorE/ScalarE:
                    # splits the copy load so neither queue's bursts gate
                    # PSUM bank recycling (GpSimd can't read PSUM).
                    if m % 2 == 0:
                        nc.vector.tensor_copy(out=dst[:, m], in_=ps)
                    else:
                        nc.scalar.copy(out=dst[:, m], in_=ps)
                    yield
            stage_qk[pr] = (qt_sb, kt_sb)

            # V: natural [s, e] layout with a ones column appended per head.
            # s=0:128 runs per batch; the two 51-row s=128:179 tails merge
            # into ONE stationary group ([b0 128:192 | b1 128:192], 2x64
            # cols) whose psum rows land at 0:51 (b0) and 64:115 (b1), so
            # both evacuations are partition-aligned.  Batch 1's jt=1 data
            # then lives at partitions 64:115 end-to-end.
            vpads_by_b = [[None, None], [None, None]]
            for bi in range(2):
                vp = v_pool.tile(
                    [128, H, D + 1], BF16, tag="vpad0", name=f"vp0_{pr}_{bi}"
                )
                nc.gpsimd.memset(vp[:, :, D : D + 1], 1.0)
                vpads_by_b[bi][0] = vp
                for n in range(2):
                    ps = pp_proj.tile(
                        [128, 512], F32, tag="proj", name=f"psv_{pr}_{bi}_0_{n}"
                    )
                    for k in range(KT):
                        nc.tensor.matmul(
                            out=ps,
                            lhsT=xt[:, k, bi, 0:128],
                            rhs=wv_sb[:, k, n * 512 : (n + 1) * 512],
                            start=(k == 0),
                            stop=(k == KT - 1),
                        )
                    nc.scalar.copy(
                        out=vp[:, n * 8 : (n + 1) * 8, 0:D],
                        in_=ps.rearrange("p (h d) -> p h d", d=D),
                    )
                    yield
            vp1 = [
                v_pool.tile([128, H, D + 1], BF16, tag="vpad1", name=f"vp1_{pr}_{bi}")
                for bi in range(2)
            ]
            nc.gpsimd.memset(vp1[0][0:S1, :, D : D + 1], 1.0)
            nc.gpsimd.memset(vp1[1][64 : 64 + S1, :, D : D + 1], 1.0)
            vpads_by_b[0][1] = vp1[0]
            vpads_by_b[1][1] = vp1[1]
            for n in range(2):
                ps = pp_proj.tile([128, 512], F32, tag="proj", name=f"psv_{pr}_t_{n}")
                for k in range(KT):
                    nc.tensor.matmul(
                        out=ps,
                        lhsT=xtail[:, k],
                        rhs=wv_sb[:, k, n * 512 : (n + 1) * 512],
                        start=(k == 0),
                        stop=(k == KT - 1),
                    )
                nc.scalar.copy(
                    out=vp1[0][0:S1, n * 8 : (n + 1) * 8, 0:D],
                    in_=ps[0:S1].rearrange("p (h d) -> p h d", d=D),
                )
                nc.scalar.copy(
                    out=vp1[1][64 : 64 + S1, n * 8 : (n + 1) * 8, 0:D],
                    in_=ps[64 : 64 + S1].rearrange("p (h d) -> p h d", d=D),
                )
                yield
            stage_v[(pr, 0)] = vpads_by_b[0]
            stage_v[(pr, 1)] = vpads_by_b[1]

        def attn_gen(pr):
            """Attention + LayerNorm for both batches of pair `pr` (batch-
            major); yields per pair-block for the projection interleave.
            PV trails scores by one head pair.  The last two pairs run in
            tail mode: their PV groups pack two heads into ONE psum bank so
            both pairs' attention can interleave at the end of the kernel
            (denser tail PE stream keeps the HAM clock gate at full rate)."""
            tail_mode = False
            qt_sb, kt_sb = stage_qk.pop(pr)
            o_by_b = []
            for bi in range(2):
                b = 2 * pr + bi
                o_by_b.append([
                    o_pool.tile([128, E], F32, tag=f"o{st}", name=f"o{st}_{b}")
                    for st, _ in enumerate(S_TILES)
                ])

            # LayerNorm blocks (VectorE stats + apply; rstd via bit-trick +
            # Newton rsqrt -- no ACT table function) are deferred: queued as
            # closures right after their batch and drained one per pair-
            # block, so the ~3us LN burst never sits ahead of the boundary
            # softmax muls in the VectorE FIFO.
            def make_ln(bi, it):
                b = 2 * pr + bi
                is_, in_n = S_TILES[it]
                o_sb = o_by_b[bi][it]

                def ln_block():
                    stats = ln_pool.tile([128, 2, 6], F32, tag="stats", name=f"st_{b}_{it}")
                    mv = ln_pool.tile([128, 2], F32, tag="mv", name=f"mv_{b}_{it}")
                    nc.vector.bn_stats(out=stats[:in_n, 0], in_=o_sb[:in_n, 0:512])
                    nc.vector.bn_stats(out=stats[:in_n, 1], in_=o_sb[:in_n, 512:E])
                    nc.vector.bn_aggr(out=mv[:in_n], in_=stats[:in_n])
                    ve = ln_pool.tile([128, 1], F32, tag="ve", name=f"ve_{b}_{it}")
                    nc.vector.tensor_scalar_add(ve[:in_n], mv[:in_n, 1:2], LN_EPS)
                    rstd = r_pool.tile([128, 1], F32, tag="rstd", name=f"rs_{b}_{it}")
                    nc.vector.tensor_scalar(
                        out=rstd[:in_n].bitcast(U32),
                        in0=ve[:in_n].bitcast(U32),
                        scalar1=1,
                        scalar2=None,
                        op0=ALU.logical_shift_right,
                    )
                    nc.vector.tensor_tensor(
                        out=rstd[:in_n].bitcast(U32),
                        in0=magic_t[:in_n],
                        in1=rstd[:in_n].bitcast(U32),
                        op=ALU.subtract,
                    )
                    t0 = r_pool.tile([128, 1], F32, tag="nt0", name=f"nt0_{b}_{it}")
                    for _ in range(2):
                        nc.vector.tensor_mul(out=t0[:in_n], in0=rstd[:in_n], in1=rstd[:in_n])
                        nc.vector.tensor_mul(out=t0[:in_n], in0=t0[:in_n], in1=ve[:in_n])
                        nc.vector.tensor_scalar(
                            out=t0[:in_n], in0=t0[:in_n],
                            scalar1=-0.5, scalar2=1.5, op0=ALU.mult, op1=ALU.add,
                        )
                        nc.vector.tensor_mul(out=rstd[:in_n], in0=rstd[:in_n], in1=t0[:in_n])
                    nc.vector.tensor_scalar(
                        out=o_sb[:in_n],
                        in0=o_sb[:in_n],
                        scalar1=mv[:in_n, 0:1],
                        scalar2=rstd[:in_n],
                        op0=ALU.subtract,
                        op1=ALU.mult,
                    )
                    if apply_gb:
                        nc.vector.tensor_mul(out=o_sb[:in_n], in0=o_sb[:in_n], in1=gamma_b[:in_n])
                        nc.vector.tensor_add(out=o_sb[:in_n], in0=o_sb[:in_n], in1=beta_b[:in_n])
                    nc.sync.dma_start(out=out[b, is_ : is_ + in_n], in_=o_sb[:in_n])

                return ln_block

            for bi in range(2):
                b = 2 * pr + bi
                yield ("need_v", pr, bi)
                vpads = stage_v.pop((pr, bi))
                o_tiles = o_by_b[bi]
                p_ts = {}
                ps_o4 = [None, None]
                ps_o2 = [None]

                def emit_scores_pair(m):
                    # Heads 2m (kt/qt rows 0:64) and 2m+1 (rows 64:128) live
                    # in disjoint PE row groups, so their LDWEIGHTS+matmuls
                    # overlap when emitted adjacently (the PE pulls a
                    # non-conflicting row_grp LDW ahead of in-flight MMs).
                    ps_pair = []
                    for hp in range(2):
                        h = 2 * m + hp
                        ps_s = pp_s.tile([128, 2, S], F32, tag="s", name=f"pss_{b}_{h}")
                        ps_pair.append(ps_s)
                    # Batch 1's jt=1 scores write psum base partition 64 (the
                    # V-tail packing layout; <=64-row outputs at base 64 are
                    # col-group aligned).
                    for it, (js, je) in enumerate(((0, 128), (128, S))):
                        ob = 64 if (bi == 1 and it == 1) else 0
                        for hp in range(2):
                            r0 = hp * D
                            jn = je - js
                            nc.tensor.matmul(
                                out=ps_pair[hp][ob : ob + jn, it],
                                lhsT=kt_sb[r0 : r0 + D, m, bi, js:je],
                                rhs=qt_sb[r0 : r0 + D, m, bi, :],
                                start=True,
                                stop=True,
                            )
                    # Multiplicative bias + exp. Stale rows j>=S of the jt=1
                    # half see wsc=0 -> p=1; excluded by the :jn PV slices.
                    # The mul evacuates PSUM to SBUF so the bank frees after
                    # the (fast) VectorE op, not the (queued) ScalarE exp.
                    for hp in range(2):
                        h = 2 * m + hp
                        ps_s = ps_pair[hp]
                        s_sb = s_pool.tile([128, 2, S], F32, tag="sf", name=f"sf_{b}_{h}")
                        nc.vector.tensor_mul(
                            out=s_sb, in0=ps_s, in1=wsc_sb[:, 2 * bi : 2 * bi + 2]
                        )
                        p_t = p_pool.tile([128, 2, S], BF16, tag="p", name=f"p_{b}_{h}")
                        nc.scalar.activation(out=p_t, in_=s_sb, func=AF.Exp)
                        p_ts[h] = p_t

                def emit_pv(h):
                    if tail_mode:
                        emit_pv_tail(h)
                        return
                    hc = h % 4
                    p_t = p_ts.pop(h)
                    # PV: 4 heads share a psum bank: [i, 4, 65] where col 64
                    # of each head is the softmax denominator (ones col in V).
                    if hc == 0:
                        ps_o4[0] = pp_o.tile([128, 4, D + 1], F32, tag="po", name=f"pso_{b}_{h}_0")
                        ps_o4[1] = pp_o.tile([128, 4, D + 1], F32, tag="po", name=f"pso_{b}_{h}_1")
                    for it, (is_, in_n) in enumerate(S_TILES):
                        for jt, (js, jn) in enumerate(S_TILES):
                            jb = 64 if (bi == 1 and jt == 1) else 0
                            nc.tensor.matmul(
                                out=ps_o4[it][:in_n, hc],
                                lhsT=p_t[jb : jb + jn, jt, is_ : is_ + in_n],
                                rhs=vpads[jt][jb : jb + jn, h],
                                start=(jt == 0),
                                stop=(jt == 1),
                            )
                    if hc == 3:
                        # Batched normalize for the 4-head group: one
                        # reciprocal of the 4 denominators, one broadcast
                        # multiply writing [i, 4*64] of the output tile.
                        g0 = (h - 3) * D
                        for it, (is_, in_n) in enumerate(S_TILES):
                            rec = r_pool.tile([128, 4], F32, tag="rec4", name=f"rc_{b}_{h}_{it}")
                            nc.vector.reciprocal(
                                out=rec[:in_n], in_=ps_o4[it][:in_n, :, D]
                            )
                            rb = rec[:in_n]
                            rbc = bass.AP(
                                tensor=rb.tensor,
                                offset=rb.offset,
                                ap=list(rb.ap) + [[0, D]],
                            )
                            nc.vector.tensor_mul(
                                out=o_tiles[it][:in_n, g0 : g0 + 4 * D].rearrange(
                                    "p (h d) -> p h d", d=D
                                ),
                                in0=ps_o4[it][:in_n, :, 0:D],
                                in1=rbc,
                            )

                def emit_pv_tail(h):
                    hc = h % 2
                    p_t = p_ts.pop(h)
                    if hc == 0:
                        ps_o2[0] = pp_o.tile(
                            [128, 2, 2, D + 1], F32, tag="po", name=f"pso2_{b}_{h}"
                        )
                    for it, (is_, in_n) in enumerate(S_TILES):
                        for jt, (js, jn) in enumerate(S_TILES):
                            jb = 64 if (bi == 1 and jt == 1) else 0
                            nc.tensor.matmul(
                                out=ps_o2[0][:in_n, it, hc],
                                lhsT=p_t[jb : jb + jn, jt, is_ : is_ + in_n],
                                rhs=vpads[jt][jb : jb + jn, h],
                                start=(jt == 0),
                                stop=(jt == 1),
                            )
                    if hc == 1:
                        g0 = (h - 1) * D
                        for it, (is_, in_n) in enumerate(S_TILES):
                            rec = r_pool.tile([128, 2], F32, tag="rec2", name=f"rc2_{b}_{h}_{it}")
                            nc.vector.reciprocal(
                                out=rec[:in_n], in_=ps_o2[0][:in_n, it, :, D]
                            )
                            rb = rec[:in_n]
                            rbc = bass.AP(
                                tensor=rb.tensor,
                                offset=rb.offset,
                                ap=list(rb.ap) + [[0, D]],
                            )
                            nc.vector.tensor_mul(
                                out=o_tiles[it][:in_n, g0 : g0 + 2 * D].rearrange(
                                    "p (h d) -> p h d", d=D
                                ),
                                in0=ps_o2[0][:in_n, it, :, 0:D],
                                in1=rbc,
                            )

                # Pair loop: scores for pair m, then PV for pair m-1 (the
                # one-pair lag gives the mul/exp chain ~a full pair block of
                # slack before its p_t is consumed as a PV stationary).  One
                # deferred LN block from the previous pair drains per block,
                # after the boundary-critical muls.
                for m in range(H // 2):
                    emit_scores_pair(m)
                    yield "h"
                    if m >= 1:
                        emit_pv(2 * (m - 1))
                        emit_pv(2 * (m - 1) + 1)
                    if pending_ln:
                        pending_ln.popleft()()
                    yield "h"
                for h in (H - 2, H - 1):
                    emit_pv(h)
                # Queue this batch's LN blocks immediately so they can drain
                # during the NEXT batch's pair-blocks (keeps the final flush
                # short -- it runs after the HAM gate drops to half clock).
                for it in range(2):
                    pending_ln.append(make_ln(bi, it))



        # Software pipeline: attention(p) interleaved with projection chunks
        # of pair p+1.  The LAST pair's batch-1 V chunks are deferred into its
        # own attention phase so the tail keeps PE filler work (otherwise the
        # final attention runs bare and HAM re-throttles the clock).
        from collections import deque

        for _ in proj_gen(0):
            pass
        pending: deque = deque()
        pending_ln: deque = deque()
        next_pair = 1

        def push_next():
            nonlocal next_pair
            if next_pair < npair:
                pending.append((next_pair, proj_gen(next_pair)))
                next_pair += 1

        def advance_one(defer_tail=False):
            while pending:
                pr0, gen = pending[0]
                if defer_tail and pr0 == npair - 1 and pr0 in stage_qk:
                    # Hold the last pair's V chunks; they drain at its own
                    # attention phase's start (need_v spin) as tail filler.
                    return False
                if next(gen, "END") == "END":
                    pending.popleft()
                    push_next()
                    continue
                return True
            return False

        push_next()
        for p in range(npair):
            ag = attn_gen(p)
            acc = 0
            defer = p < npair - 1
            for tok in ag:
                # Distribute proj chunks: 24/32 per yield (2 yields per
                # pair-block x 8 blocks x 2 batches = 32 per phase, exactly
                # proj(p+1)'s 22 groups with slack).
                if isinstance(tok, tuple):
                    _, rp, rbi = tok
                    while (rp, rbi) not in stage_v:
                        if not advance_one():
                            break
                else:
                    acc += 24
                    while acc >= 32:
                        if not advance_one(defer):
                            break
                        acc -= 32
            # Boundary: proj(p+1) QK must be emitted before attention(p+1).
            while (p + 1) < npair and (p + 1) not in stage_qk:
                if not advance_one():
                    break
        # Flush the last batches' deferred LayerNorm blocks.
        while pending_ln:
            pending_ln.popleft()()


_NC_CACHE: dict = {}


def _get_nc(bpc: int = BPC, apply_gb: bool = True) -> bass.Bass:
    key = (bpc, apply_gb)
    if key not in _NC_CACHE:
        _NC_CACHE[key] = _build_kernel(bpc, apply_gb)
    return _NC_CACHE[key]


def _host_inputs(x, Wq, Wk, Wv, gamma, beta):
    import ml_dtypes

    bf16 = ml_dtypes.bfloat16
    x = np.asarray(x, dtype=np.float32)
    xT = np.ascontiguousarray(x.transpose(0, 2, 1)).astype(bf16)  # [B, E, S]
    idx = np.arange(S, dtype=np.float32)
    w_full = (np.abs(idx[None, :] - idx[:, None]) / S * SCALE).astype(np.float32)
    # Planes (0,1): batch 0 ([j mod 128, jt, i], zeros for j >= S).
    # Planes (2,3): batch 1, whose jt=1 rows live at partitions 64:115.
    wsc = np.zeros((128, 4, S), dtype=np.float32)
    wsc[0:128, 0] = w_full[0:128]
    wsc[0:S1, 1] = w_full[128:S]
    wsc[0:128, 2] = w_full[0:128]
    wsc[64 : 64 + S1, 3] = w_full[128:S]
    common = {
        "wqT": np.ascontiguousarray(np.asarray(Wq, np.float32).T).astype(bf16),
        "wkT": np.ascontiguousarray(np.asarray(Wk, np.float32).T).astype(bf16),
        "wvT": np.ascontiguousarray(np.asarray(Wv, np.float32).T).astype(bf16),
        "wsc": wsc,
        "gamma": np.asarray(gamma, np.float32),
        "beta": np.asarray(beta, np.float32),
    }
    return xT, common


def run(inputs: dict, trace: bool = False, trace_dir: str | None = None):
    """Run the SPMD kernel on 8 cores. Returns (full_output, exec_time_ns)."""
    xT, common = _host_inputs(**inputs)
    in_maps = [
        {**common, "xT": np.ascontiguousarray(xT[c * BPC : (c + 1) * BPC])}
        for c in range(NCORES)
    ]
    apply_gb = not (
        np.all(np.asarray(inputs["gamma"]) == 1.0)
        and np.all(np.asarray(inputs["beta"]) == 0.0)
    )
    nc = _get_nc(BPC, apply_gb)
    res = run_bass_kernel_spmd(
        nc, in_maps, core_ids=list(range(NCORES)), trace=trace, tmpdir=trace_dir
    )
    full = np.concatenate([res.results[c]["out"] for c in range(NCORES)], axis=0)
    return full.astype(np.float32), res.exec_time_ns


def kernel(x, Wq, Wk, Wv, gamma, beta):
    full, _ = run(dict(x=x, Wq=Wq, Wk=Wk, Wv=Wv, gamma=gamma, beta=beta))
    return full



# revision 68
# speedup vs baseline: 1.0081x; 1.0081x over previous
"""Trainium2 Bass kernel for nn_Attention_6468220748045.

Computes, per batch item: QKV projection -> per-head scaled attention with a
multiplicative positional bias w[i,j] = |i-j|/S -> softmax -> attn @ V ->
LayerNorm over the embedding dim.

Sharding: pure data-parallel over batch. B=128 splits as 16 batch items per
core across 8 NeuronCores; no collectives needed. Inputs are pre-laid-out on
host: x is passed transposed per batch ([B, E, S]) so both projection
orientations stream directly from SBUF, and the weights are passed transposed
([e_in, e_out]) to serve as matmul stationary operands.

Schedule (from HW trace analysis; v2 at ~507-512us, from 560us):

* Scores run in ROW-GROUP PAIRS: heads 2m / 2m+1 live at kt/qt partitions
  0:64 / 64:128, i.e. disjoint PE row groups, so emitting their four score
  matmuls adjacently lets the PE overlap their LDWEIGHTS and streams
  (observed 4ns stagger; ~374ns for all four vs ~712ns serialized).
* V-TAIL PACKING: the two 51-row s=128:179 V-projection stationary groups
  per pair merge into ONE 128-col strided stationary [b0-tail 51 | 13 pad |
  b1-tail 51 | 13 pad] (read from a separately-DMA'd packed xtail tile), so
  V-psum rows land at 0:51 / 64:115 and both evacuations stay partition-
  aligned.  Batch 1's jt=1 data therefore lives at partitions 64:115
  end-to-end (scores jt=1 write psum base 64, wsc planes 2/3, PV jt=1
  slices both operands at base 64).  Cuts V-proj from 64 to 48 N=512
  matmuls per pair.
* Projections of pair p+1 interleave into the attention of pair p (24/32
  proj psum-groups per pair-block yield); the last pair's V chunks drain
  inside its own attention phase as tail filler.
* The softmax bias-mul (VectorE) writes PSUM->SBUF staging, so score banks
  recycle after the fast VectorE op instead of the queued ScalarE exp;
  QK-projection evacuations alternate VectorE/ScalarE so neither queue's
  bursts gate PSUM recycling; V evacuations and exp stay on ScalarE.
* LayerNorm blocks are deferred closures drained one per pair-block, so
  the LN burst never sits ahead of boundary softmax muls in the VectorE
  FIFO.  rstd uses a bit-trick + Newton rsqrt (no ACT table function).

HW facts driving the above (measured on this toolchain, ldw-opt=false):
LDWEIGHTS costs ~cols/1.2 ns and only hides under a PRECEDING stream that
is long enough (proj N=358/512 hide it; attention N=65/179 pay it almost
fully); matmuls on disjoint row/col groups run concurrently; the HAM clock
gate drops the core to half clock within ~5us of the last projection
matmul -- attention-only streams never sustain it, so the tail (~35us of
work) runs at k=4/8 regardless of interleaving (a 2-pair interleaved tail
measured WORSE); DMA xbar transposes move ~13-20 GB/s so a V-stationary PV
(transposed output) loses ~5x more on transposes than it saves on the PE.

Known remaining headroom: (a) ~40us tail at half clock (HAM-gated, needs
proj-class matmuls to lift); (b) ~41us of proj chain-start waits on
evacuation queues (pp_proj=2 banks); (c) PV LDWEIGHTS serialization
(~130us wall for 28us of streams) -- no restructure found that beats the
transpose/normalize costs it would induce.
"""

import numpy as np

import concourse.bass as bass
import concourse.tile as tile
from concourse import bacc, mybir
from concourse.bass_utils import run_bass_kernel_spmd

# Problem constants (hardcoded per the self-contained-kernel contract).
B, S, E, H, D = 128, 179, 1024, 16, 64
NCORES = 8
BPC = B // NCORES          # batches per core = 16
NPAIR = BPC // 2           # batch pairs per core = 8
KT = E // 128              # contraction tiles over e_in = 8
MT = E // 128              # output tiles over e_out = 8
S0 = 128                   # first s-tile size
S1 = S - S0                # second s-tile size = 51
SP = 192                   # padded per-batch s stride in xt (V-tail packing)
S_TILES = ((0, S0), (S0, S1))
LN_EPS = 1e-5
SCALE = float(E) ** -0.5
PV_LAG = 2                 # heads by which PV trails scores

F32 = mybir.dt.float32
BF16 = mybir.dt.bfloat16
U32 = mybir.dt.uint32

AF = mybir.ActivationFunctionType
ALU = mybir.AluOpType


def _build_kernel(bpc: int = BPC, apply_gb: bool = True) -> bass.Bass:
    npair = bpc // 2
    nc = bacc.Bacc()

    xT = nc.dram_tensor("xT", [bpc, E, S], BF16, kind="ExternalInput").ap()
    wqT = nc.dram_tensor("wqT", [E, E], BF16, kind="ExternalInput").ap()
    wkT = nc.dram_tensor("wkT", [E, E], BF16, kind="ExternalInput").ap()
    wvT = nc.dram_tensor("wvT", [E, E], BF16, kind="ExternalInput").ap()
    wsc = nc.dram_tensor("wsc", [128, 4, S], F32, kind="ExternalInput").ap()
    gamma = nc.dram_tensor("gamma", [E], F32, kind="ExternalInput").ap()
    beta = nc.dram_tensor("beta", [E], F32, kind="ExternalInput").ap()
    out = nc.dram_tensor("out", [bpc, S, E], F32, kind="ExternalOutput").ap()

    with tile.TileContext(nc) as tc:
        _emit(tc, npair, out, xT, wqT, wkT, wvT, wsc, gamma, beta, apply_gb)
    nc.compile()
    return nc


def _emit(tc, npair, out, xT, wqT, wkT, wvT, wsc, gamma, beta, apply_gb):
    nc = tc.nc
    from contextlib import ExitStack

    with ExitStack() as ctx:
        singles = ctx.enter_context(tc.tile_pool(name="singles", bufs=1))
        xt_pool = ctx.enter_context(tc.tile_pool(name="xt", bufs=3))
        qk_pool = ctx.enter_context(tc.tile_pool(name="qk", bufs=3))
        v_pool = ctx.enter_context(tc.tile_pool(name="v", bufs=6))
        p_pool = ctx.enter_context(tc.tile_pool(name="p", bufs=8))
        o_pool = ctx.enter_context(tc.tile_pool(name="o", bufs=4))
        ln_pool = ctx.enter_context(tc.tile_pool(name="ln", bufs=4))
        r_pool = ctx.enter_context(tc.tile_pool(name="r", bufs=8))
        s_pool = ctx.enter_context(tc.tile_pool(name="s", bufs=4))

        # PSUM (8 banks, every tile pads to one bank):
        # proj 2 + scores 4 + PV 2 = 8.  Scores pairs use 2 banks at a time;
        # the extra pair of banks drains through the mul/exp chain of the
        # previous pair without stalling the next pair's score matmuls.
        pp_proj = ctx.enter_context(tc.tile_pool(name="pp_proj", bufs=2, space="PSUM"))
        pp_s = ctx.enter_context(tc.tile_pool(name="pp_s", bufs=4, space="PSUM"))
        pp_o = ctx.enter_context(tc.tile_pool(name="pp_o", bufs=2, space="PSUM"))

        # --- resident tensors -------------------------------------------------
        # Weight tiles: [e_in partition, k-tile, e_out]. DMA order matters for
        # startup latency: wq first, then pair-0's x.T, then wk/wv.
        xsrc = xT.rearrange("b (k p) s -> k p b s", p=128)  # [KT, 128, bpc, S]
        w_sbs = []
        for name in ("wq", "wk", "wv"):
            w_sb = singles.tile([128, KT, E], BF16, tag=f"w_{name}", name=f"w_{name}")
            w_sbs.append(w_sb)
        wq_sb, wk_sb, wv_sb = w_sbs
        # The merged V-tail stationary reads a packed [b0-tail | b1-tail]
        # tile ([128, KT, 2, 64]: 51 data + 13 zero-pad cols per batch) so
        # its psum rows land at 0:51 / 64:115; the tails are DMA'd a second
        # time from DRAM in this layout.
        xt0 = xt_pool.tile([128, KT, 2, S], BF16, tag="xt", name="xt_0")
        xtail0 = xt_pool.tile([128, KT, 2, 64], BF16, tag="xtail", name="xtail_0")
        nc.gpsimd.memset(xtail0[:, :, :, S1:64], 0.0)
        src = wqT.rearrange("(k p) e -> k p e", p=128)
        for k in range(KT):
            nc.sync.dma_start(out=wq_sb[:, k], in_=src[k])
            nc.sync.dma_start(out=xt0[:, k], in_=xsrc[k, :, 0:2, :])
            nc.sync.dma_start(out=xtail0[:, k, :, 0:S1], in_=xsrc[k, :, 0:2, 128:S])
        for w_sb, wap in ((wk_sb, wkT), (wv_sb, wvT)):
            src = wap.rearrange("(k p) e -> k p e", p=128)
            for k in range(KT):
                nc.sync.dma_start(out=w_sb[:, k], in_=src[k])

        # Positional bias (already includes softmax scale), host-precomputed.
        # Planes (0,1) serve batch 0 of a pair ([j mod 128, jt, i], zero rows
        # for j >= S); planes (2,3) serve batch 1, whose jt=1 rows live at
        # partitions 64:115 (V-tail packing layout).
        wsc_sb = singles.tile([128, 4, S], F32, tag="wsc")
        nc.sync.dma_start(out=wsc_sb, in_=wsc)

        if apply_gb:
            gamma_b = singles.tile([128, E], F32, tag="gamma")
            beta_b = singles.tile([128, E], F32, tag="beta")
            nc.sync.dma_start(
                out=gamma_b,
                in_=bass.AP(tensor=gamma.tensor, offset=gamma.offset, ap=[[0, 128]] + gamma.ap),
            )
            nc.sync.dma_start(
                out=beta_b,
                in_=bass.AP(tensor=beta.tensor, offset=beta.offset, ap=[[0, 128]] + beta.ap),
            )
        # Magic constant for the bit-trick rsqrt seed (no ACT table needed).
        magic_t = singles.tile([128, 1], U32, tag="magic")
        nc.vector.memset(magic_t, 0x5F3759DF)

        # Per-pair SBUF products handed from the projection stage to the
        # attention stage (software pipeline).  stage_qk lands after the QK
        # phase; stage_v[(pr, bi)] after that batch's V chunks, so the last
        # pair's V work can interleave into its own attention phase.
        stage_qk: dict = {}
        stage_v: dict = {}

        def proj_gen(pr):
            """QKV projections for batch pair `pr`; yields after each PE
            psum-group (~8 matmuls) so attention of pair pr-1 interleaves at
            chunk granularity (the PE weight-load path is one-deep, so finer
            interleave only serializes LDWEIGHTS of the two streams)."""
            if pr == 0:
                xt, xtail = xt0, xtail0
            else:
                xt = xt_pool.tile([128, KT, 2, S], BF16, tag="xt", name=f"xt_{pr}")
                xtail = xt_pool.tile(
                    [128, KT, 2, 64], BF16, tag="xtail", name=f"xtail_{pr}"
                )
                nc.gpsimd.memset(xtail[:, :, :, S1:64], 0.0)
                for k in range(KT):
                    nc.sync.dma_start(
                        out=xt[:, k], in_=xsrc[k, :, 2 * pr : 2 * pr + 2, :]
                    )
                    nc.sync.dma_start(
                        out=xtail[:, k, :, 0:S1],
                        in_=xsrc[k, :, 2 * pr : 2 * pr + 2, 128:S],
                    )

            # Q.T / K.T: out[e_out, s2], s2 = 2*S = 358 (both batches at once).
            qt_sb = qk_pool.tile([128, MT, 2, S], BF16, tag="qt", name=f"qt_{pr}")
            kt_sb = qk_pool.tile([128, MT, 2, S], BF16, tag="kt", name=f"kt_{pr}")
            for w_sb, dst in ((wq_sb, qt_sb), (wk_sb, kt_sb)):
                for m in range(MT):
                    ps = pp_proj.tile([128, 2, S], F32, tag="proj", name=f"psqk_{pr}_{m}")
                    for k in range(KT):
                        nc.tensor.matmul(
                            out=ps,
                            lhsT=w_sb[:, k, m * 128 : (m + 1) * 128],
                            rhs=xt[:, k],
                            start=(k == 0),
                            stop=(k == KT - 1),
                        )
                    # PSUM->SBUF evacuation alternating VectorE/ScalarE:
                    # splits the copy load so neither queue's bursts gate
                    # PSUM bank recycling (GpSimd can't read PSUM).
                    if m % 2 == 0:
                        nc.vector.tensor_copy(out=dst[:, m], in_=ps)
                    else:
                        nc.scalar.copy(out=dst[:, m], in_=ps)
                    yield
            stage_qk[pr] = (qt_sb, kt_sb)

            # V: natural [s, e] layout with a ones column appended per head.
            # s=0:128 runs per batch; the two 51-row s=128:179 tails merge
            # into ONE stationary group ([b0 128:192 | b1 128:192], 2x64
            # cols) whose psum rows land at 0:51 (b0) and 64:115 (b1), so
            # both evacuations are partition-aligned.  Batch 1's jt=1 data
            # then lives at partitions 64:115 end-to-end.
            vpads_by_b = [[None, None], [None, None]]
            for bi in range(2):
                vp = v_pool.tile(
                    [128, H, D + 1], BF16, tag="vpad0", name=f"vp0_{pr}_{bi}"
                )
                nc.gpsimd.memset(vp[:, :, D : D + 1], 1.0)
                vpads_by_b[bi][0] = vp
                for n in range(2):
                    ps = pp_proj.tile(
                        [128, 512], F32, tag="proj", name=f"psv_{pr}_{bi}_0_{n}"
                    )
                    for k in range(KT):
                        nc.tensor.matmul(
                            out=ps,
                            lhsT=xt[:, k, bi, 0:128],
                            rhs=wv_sb[:, k, n * 512 : (n + 1) * 512],
                            start=(k == 0),
                            stop=(k == KT - 1),
                        )
                    nc.scalar.copy(
                        out=vp[:, n * 8 : (n + 1) * 8, 0:D],
                        in_=ps.rearrange("p (h d) -> p h d", d=D),
                    )
                    yield
            vp1 = [
                v_pool.tile([128, H, D + 1], BF16, tag="vpad1", name=f"vp1_{pr}_{bi}")
                for bi in range(2)
            ]
            nc.gpsimd.memset(vp1[0][0:S1, :, D : D + 1], 1.0)
            nc.gpsimd.memset(vp1[1][64 : 64 + S1, :, D : D + 1], 1.0)
            vpads_by_b[0][1] = vp1[0]
            vpads_by_b[1][1] = vp1[1]
            for n in range(2):
                ps = pp_proj.tile([128, 512], F32, tag="proj", name=f"psv_{pr}_t_{n}")
                for k in range(KT):
                    nc.tensor.matmul(
                        out=ps,
                        lhsT=xtail[:, k],
                        rhs=wv_sb[:, k, n * 512 : (n + 1) * 512],
                        start=(k == 0),
                        stop=(k == KT - 1),
                    )
                nc.scalar.copy(
                    out=vp1[0][0:S1, n * 8 : (n + 1) * 8, 0:D],
                    in_=ps[0:S1].rearrange("p (h d) -> p h d", d=D),
                )
                nc.scalar.copy(
                    out=vp1[1][64 : 64 + S1, n * 8 : (n + 1) * 8, 0:D],
                    in_=ps[64 : 64 + S1].rearrange("p (h d) -> p h d", d=D),
                )
                yield
            stage_v[(pr, 0)] = vpads_by_b[0]
            stage_v[(pr, 1)] = vpads_by_b[1]

        def attn_gen(pr):
            """Attention + LayerNorm for both batches of pair `pr` (batch-
            major); yields per pair-block for the projection interleave.
            PV trails scores by one head pair.  The last two pairs run in
            tail mode: their PV groups pack two heads into ONE psum bank so
            both pairs' attention can interleave at the end of the kernel
            (denser tail PE stream keeps the HAM clock gate at full rate)."""
            tail_mode = False
            qt_sb, kt_sb = stage_qk.pop(pr)
            o_by_b = []
            for bi in range(2):
                b = 2 * pr + bi
                o_by_b.append([
                    o_pool.tile([128, E], F32, tag=f"o{st}", name=f"o{st}_{b}")
                    for st, _ in enumerate(S_TILES)
                ])

            # LayerNorm blocks (VectorE stats + apply; rstd via bit-trick +
            # Newton rsqrt -- no ACT table function) are deferred: queued as
            # closures right after their batch and drained one per pair-
            # block, so the ~3us LN burst never sits ahead of the boundary
            # softmax muls in the VectorE FIFO.
            def make_ln(bi, it):
                b = 2 * pr + bi
                is_, in_n = S_TILES[it]
                o_sb = o_by_b[bi][it]

                def ln_block():
                    stats = ln_pool.tile([128, 2, 6], F32, tag="stats", name=f"st_{b}_{it}")
                    mv = ln_pool.tile([128, 2], F32, tag="mv", name=f"mv_{b}_{it}")
                    nc.vector.bn_stats(out=stats[:in_n, 0], in_=o_sb[:in_n, 0:512])
                    nc.vector.bn_stats(out=stats[:in_n, 1], in_=o_sb[:in_n, 512:E])
                    nc.vector.bn_aggr(out=mv[:in_n], in_=stats[:in_n])
                    ve = ln_pool.tile([128, 1], F32, tag="ve", name=f"ve_{b}_{it}")
                    nc.vector.tensor_scalar_add(ve[:in_n], mv[:in_n, 1:2], LN_EPS)
                    rstd = r_pool.tile([128, 1], F32, tag="rstd", name=f"rs_{b}_{it}")
                    nc.vector.tensor_scalar(
                        out=rstd[:in_n].bitcast(U32),
                        in0=ve[:in_n].bitcast(U32),
                        scalar1=1,
                        scalar2=None,
                        op0=ALU.logical_shift_right,
                    )
                    nc.vector.tensor_tensor(
                        out=rstd[:in_n].bitcast(U32),
                        in0=magic_t[:in_n],
                        in1=rstd[:in_n].bitcast(U32),
                        op=ALU.subtract,
                    )
                    t0 = r_pool.tile([128, 1], F32, tag="nt0", name=f"nt0_{b}_{it}")
                    for _ in range(2):
                        nc.vector.tensor_mul(out=t0[:in_n], in0=rstd[:in_n], in1=rstd[:in_n])
                        nc.vector.tensor_mul(out=t0[:in_n], in0=t0[:in_n], in1=ve[:in_n])
                        nc.vector.tensor_scalar(
                            out=t0[:in_n], in0=t0[:in_n],
                            scalar1=-0.5, scalar2=1.5, op0=ALU.mult, op1=ALU.add,
                        )
                        nc.vector.tensor_mul(out=rstd[:in_n], in0=rstd[:in_n], in1=t0[:in_n])
                    nc.vector.tensor_scalar(
                        out=o_sb[:in_n],
                        in0=o_sb[:in_n],
                        scalar1=mv[:in_n, 0:1],
                        scalar2=rstd[:in_n],
                        op0=ALU.subtract,
                        op1=ALU.mult,
                    )
                    if apply_gb:
                        nc.vector.tensor_mul(out=o_sb[:in_n], in0=o_sb[:in_n], in1=gamma_b[:in_n])
                        nc.vector.tensor_add(out=o_sb[:in_n], in0=o_sb[:in_n], in1=beta_b[:in_n])
                    nc.sync.dma_start(out=out[b, is_ : is_ + in_n], in_=o_sb[:in_n])

                return ln_block

            for bi in range(2):
                b = 2 * pr + bi
                yield ("need_v", pr, bi)
                vpads = stage_v.pop((pr, bi))
                o_tiles = o_by_b[bi]
                p_ts = {}
                ps_o4 = [None, None]
                ps_o2 = [None]

                def emit_scores_pair(m):
                    # Heads 2m (kt/qt rows 0:64) and 2m+1 (rows 64:128) live
                    # in disjoint PE row groups, so their LDWEIGHTS+matmuls
                    # overlap when emitted adjacently (the PE pulls a
                    # non-conflicting row_grp LDW ahead of in-flight MMs).
                    ps_pair = []
                    for hp in range(2):
                        h = 2 * m + hp
                        ps_s = pp_s.tile([128, 2, S], F32, tag="s", name=f"pss_{b}_{h}")
                        ps_pair.append(ps_s)
                    # Batch 1's jt=1 scores write psum base partition 64 (the
                    # V-tail packing layout; <=64-row outputs at base 64 are
                    # col-group aligned).
                    for it, (js, je) in enumerate(((0, 128), (128, S))):
                        ob = 64 if (bi == 1 and it == 1) else 0
                        for hp in range(2):
                            r0 = hp * D
                            jn = je - js
                            nc.tensor.matmul(
                                out=ps_pair[hp][ob : ob + jn, it],
                                lhsT=kt_sb[r0 : r0 + D, m, bi, js:je],
                                rhs=qt_sb[r0 : r0 + D, m, bi, :],
                                start=True,
                                stop=True,
                            )
                    # Multiplicative bias + exp. Stale rows j>=S of the jt=1
                    # half see wsc=0 -> p=1; excluded by the :jn PV slices.
                    # The mul evacuates PSUM to SBUF so the bank frees after
                    # the (fast) VectorE op, not the (queued) ScalarE exp.
                    for hp in range(2):
                        h = 2 * m + hp
                        ps_s = ps_pair[hp]
                        s_sb = s_pool.tile([128, 2, S], F32, tag="sf", name=f"sf_{b}_{h}")
                        nc.vector.tensor_mul(
                            out=s_sb, in0=ps_s, in1=wsc_sb[:, 2 * bi : 2 * bi + 2]
                        )
                        p_t = p_pool.tile([128, 2, S], BF16, tag="p", name=f"p_{b}_{h}")
                        nc.scalar.activation(out=p_t, in_=s_sb, func=AF.Exp)
                        p_ts[h] = p_t

                def emit_pv(h):
                    if tail_mode:
                        emit_pv_tail(h)
                        return
                    hc = h % 4
                    p_t = p_ts.pop(h)
                    # PV: 4 heads share a psum bank: [i, 4, 65] where col 64
                    # of each head is the softmax denominator (ones col in V).
                    if hc == 0:
                        ps_o4[0] = pp_o.tile([128, 4, D + 1], F32, tag="po", name=f"pso_{b}_{h}_0")
                        ps_o4[1] = pp_o.tile([128, 4, D + 1], F32, tag="po", name=f"pso_{b}_{h}_1")
                    for it, (is_, in_n) in enumerate(S_TILES):
                        for jt, (js, jn) in enumerate(S_TILES):
                            jb = 64 if (bi == 1 and jt == 1) else 0
                            nc.tensor.matmul(
                                out=ps_o4[it][:in_n, hc],
                                lhsT=p_t[jb : jb + jn, jt, is_ : is_ + in_n],
                                rhs=vpads[jt][jb : jb + jn, h],
                                start=(jt == 0),
                                stop=(jt == 1),
                            )
                    if hc == 3:
                        # Batched normalize for the 4-head group: one
                        # reciprocal of the 4 denominators, one broadcast
                        # multiply writing [i, 4*64] of the output tile.
                        g0 = (h - 3) * D
                        for it, (is_, in_n) in enumerate(S_TILES):
                            rec = r_pool.tile([128, 4], F32, tag="rec4", name=f"rc_{b}_{h}_{it}")
                            nc.vector.reciprocal(
                                out=rec[:in_n], in_=ps_o4[it][:in_n, :, D]
                            )
                            rb = rec[:in_n]
                            rbc = bass.AP(
                                tensor=rb.tensor,
                                offset=rb.offset,
                                ap=list(rb.ap) + [[0, D]],
                            )
                            nc.vector.tensor_mul(
                                out=o_tiles[it][:in_n, g0 : g0 + 4 * D].rearrange(
                                    "p (h d) -> p h d", d=D
                                ),
                                in0=ps_o4[it][:in_n, :, 0:D],
                                in1=rbc,
                            )

                def emit_pv_tail(h):
                    hc = h % 2
                    p_t = p_ts.pop(h)
                    if hc == 0:
                        ps_o2[0] = pp_o.tile(
                            [128, 2, 2, D + 1], F32, tag="po", name=f"pso2_{b}_{h}"
                        )
                    for it, (is_, in_n) in enumerate(S_TILES):
                        for jt, (js, jn) in enumerate(S_TILES):
                            jb = 64 if (bi == 1 and jt == 1) else 0
                            nc.tensor.matmul(
                                out=ps_o2[0][:in_n, it, hc],
                                lhsT=p_t[jb : jb + jn, jt, is_ : is_ + in_n],
                                rhs=vpads[jt][jb : jb + jn, h],
                                start=(jt == 0),
                                stop=(jt == 1),
                            )
                    if hc == 1:
                        g0 = (h - 1) * D
                        for it, (is_, in_n) in enumerate(S_TILES):
                            rec = r_pool.tile([128, 2], F32, tag="rec2", name=f"rc2_{b}_{h}_{it}")
                            nc.vector.reciprocal(
                                out=rec[:in_n], in_=ps_o2[0][:in_n, it, :, D]
                            )
                            rb = rec[:in_n]
                            rbc = bass.AP(
                                tensor=rb.tensor,
                                offset=rb.offset,
                                ap=list(rb.ap) + [[0, D]],
                            )
                            nc.vector.tensor_mul(
                                out=o_tiles[it][:in_n, g0 : g0 + 2 * D].rearrange(
                                    "p (h d) -> p h d", d=D
                                ),
                                in0=ps_o2[0][:in_n, it, :, 0:D],
                                in1=rbc,
                            )

                # Pair loop: scores for pair m, then PV for pair m-1 (the
                # one-pair lag gives the mul/exp chain ~a full pair block of
                # slack before its p_t is consumed as a PV stationary).  One
                # deferred LN block from the previous pair drains per block,
                # after the boundary-critical muls.
                for m in range(H // 2):
                    emit_scores_pair(m)
                    yield "h"
                    if m >= 1:
                        emit_pv(2 * (m - 1))
                        emit_pv(2 * (m - 1) + 1)
                    if pending_ln:
                        pending_ln.popleft()()
                    yield "h"
                for h in (H - 2, H - 1):
                    emit_pv(h)
                # Queue this batch's LN blocks immediately so they can drain
                # during the NEXT batch's pair-blocks (keeps the final flush
                # short -- it runs after the HAM gate drops to half clock).
                for it in range(2):
                    pending_ln.append(make_ln(bi, it))



        # Software pipeline: attention(p) interleaved with projection chunks
        # of pair p+1.  The LAST pair's batch-1 V chunks are deferred into its
        # own attention phase so the tail keeps PE filler work (otherwise the
        # final attention runs bare and HAM re-throttles the clock).
        from collections import deque

        for _ in proj_gen(0):
            pass
        pending: deque = deque()
        pending_ln: deque = deque()
        next_pair = 1

        def push_next():
            nonlocal next_pair
            if next_pair < npair:
                pending.append((next_pair, proj_gen(next_pair)))
                next_pair += 1

        def advance_one(defer_tail=False):
            while pending:
                pr0, gen = pending[0]
                if defer_tail and pr0 == npair - 1 and pr0 in stage_qk:
                    # Hold the last pair's V chunks; they drain at its own
                    # attention phase's start (need_v spin) as tail filler.
                    return False
                if next(gen, "END") == "END":
                    pending.popleft()
                    push_next()
                    continue
                return True
            return False

        push_next()
        for p in range(npair):
            ag = attn_gen(p)
            acc = 0
            defer = p < npair - 1
            for tok in ag:
                # Distribute proj chunks: 24/32 per yield (2 yields per
                # pair-block x 8 blocks x 2 batches = 32 per phase, exactly
                # proj(p+1)'s 22 groups with slack).
                if isinstance(tok, tuple):
                    _, rp, rbi = tok
                    while (rp, rbi) not in stage_v:
                        if not advance_one():
                            break
                else:
                    acc += 24
                    while acc >= 32:
                        if not advance_one(defer):
                            break
                        acc -= 32
            # Boundary: proj(p+1) QK must be emitted before attention(p+1).
            while (p + 1) < npair and (p + 1) not in stage_qk:
                if not advance_one():
                    break
        # Flush the last batches' deferred LayerNorm blocks.
        while pending_ln:
            pending_ln.popleft()()


_NC_CACHE: dict = {}


def _get_nc(bpc: int = BPC, apply_gb: bool = True) -> bass.Bass:
    key = (bpc, apply_gb)
    if key not in _NC_CACHE:
        _NC_CACHE[key] = _build_kernel(bpc, apply_gb)
    return _NC_CACHE[key]


def _host_inputs(x, Wq, Wk, Wv, gamma, beta):
    import ml_dtypes

    bf16 = ml_dtypes.bfloat16
    x = np.asarray(x, dtype=np.float32)
    xT = np.ascontiguousarray(x.transpose(0, 2, 1)).astype(bf16)  # [B, E, S]
    idx = np.arange(S, dtype=np.float32)
    w_full = (np.abs(idx[None, :] - idx[:, None]) / S * SCALE).astype(np.float32)
    # Planes (0,1): batch 0 ([j mod 128, jt, i], zeros for j >= S).
    # Planes (2,3): batch 1, whose jt=1 rows live at partitions 64:115.
    wsc = np.zeros((128, 4, S), dtype=np.float32)
    wsc[0:128, 0] = w_full[0:128]
    wsc[0:S1, 1] = w_full[128:S]
    wsc[0:128, 2] = w_full[0:128]
    wsc[64 : 64 + S1, 3] = w_full[128:S]
    common = {
        "wqT": np.ascontiguousarray(np.asarray(Wq, np.float32).T).astype(bf16),
        "wkT": np.ascontiguousarray(np.asarray(Wk, np.float32).T).astype(bf16),
        "wvT": np.ascontiguousarray(np.asarray(Wv, np.float32).T).astype(bf16),
        "wsc": wsc,
        "gamma": np.asarray(gamma, np.float32),
        "beta": np.asarray(beta, np.float32),
    }
    return xT, common


def run(inputs: dict, trace: bool = False, trace_dir: str | None = None):
    """Run the SPMD kernel on 8 cores. Returns (full_output, exec_time_ns)."""
    xT, common = _host_inputs(**inputs)
    in_maps = [
        {**common, "xT": np.ascontiguousarray(xT[c * BPC : (c + 1) * BPC])}
        for c in range(NCORES)
    ]
    apply_gb = not (
        np.all(np.asarray(inputs["gamma"]) == 1.0)
        and np.all(np.asarray(inputs["beta"]) == 0.0)
    )
    nc = _get_nc(BPC, apply_gb)
    res = run_bass_kernel_spmd(
        nc, in_maps, core_ids=list(range(NCORES)), trace=trace, tmpdir=trace_dir
    )
    full = np.concatenate([res.results[c]["out"] for c in range(NCORES)], axis=0)
    return full.astype(np.float32), res.exec_time_ns


def kernel(x, Wq, Wk, Wv, gamma, beta):
    full, _ = run(dict(x=x, Wq=Wq, Wk=Wk, Wv=Wv, gamma=gamma, beta=beta))
    return full

